# revision 23
# baseline (speedup 1.0000x reference)
"""Banded additive (Bahdanau) attention on 8 TRN2 NeuronCores.

Reference computation (B=2, L=1024, D=256, U=32, width 128, keys j in [i-64, i+63]):
    q = x @ Wt; k = x @ Wx
    e[b,i,j] = exp(Wa . tanh(q_i + k_j + bh) + ba) * band(i,j)
    v = (e / (sum_j e + eps)) @ x

Sharding: core = b*4 + chunk handles batch b, queries [chunk*256, chunk*256+256).
Each core receives a 384-row halo of x (queries +-64), so the band is fully
local: no collectives.

Raw Bass with explicit semaphores (this walrus build encodes only ONE
sync-wait per instruction, so Tile's attached waits don't compile; standalone
wait_ge instructions are unlimited).

v2 structure: scoring runs in two i-half passes (queries [0,128) then
[128,256)); the entire softmax/scatter/transpose/value tail of half 0
overlaps half 1's scoring. Constants arrive in two parallel DMAs (PE-critical
operands first), and ACT's tanh/exp spline tables are preloaded with dummy
activations during the load.
"""

import sys

for _p in ("/opt/trn_rl_repo",):
    if _p not in sys.path:
        sys.path.insert(0, _p)

import ml_dtypes
import numpy as np

import concourse.bass as bass
from concourse import mybir
from concourse.bass_utils import run_bass_kernel_spmd

B, L, D, U = 2, 1024, 256, 32
W = 128            # attention width (band)
CHUNK = 256        # queries per core
NH = 384           # halo'd key rows per core: 64 + 256 + 64
G = 4              # unit-replication groups (4*32 = 128 partitions)
NBLK = W // G      # 32 offset blocks of 4
BPC = 8            # offset blocks per DVE/ACT op within a half-pass
NCC = NBLK // BPC  # 4 chunks per half-pass
EPS = 1e-7

BF = mybir.dt.bfloat16
F32 = mybir.dt.float32
NPBF = ml_dtypes.bfloat16

# blob A columns (PE-critical): xT 2x384, wt4 2x128, wx4 2x128
A_XT = 0
A_WT = A_XT + 2 * NH
A_WX = A_WT + 2 * 128
TOTA = A_WX + 2 * 128
# blob B columns: xh 3x256, wa4 4, mask 2x128, bh (f32 as 2 bf16)
B_XH = 0
B_WA = B_XH + 3 * D
B_MASK = B_WA + G
B_BH = B_MASK + 2 * W
TOTB = B_BH + 2


def build_bass():
    nc = bass.Bass()
    blobA = nc.declare_dram_parameter("blobA", [128, TOTA], BF, isOutput=False)
    blobB = nc.declare_dram_parameter("blobB", [128, TOTB], BF, isOutput=False)
    out = nc.declare_dram_parameter("out", [CHUNK, D], F32, isOutput=True)
    dense = nc.dram_tensor("dense", [CHUNK, NH], BF)

    ctxs = []

    def sb(name, shape, dtype=BF):
        cm = nc.sbuf_tensor(name, shape, dtype)
        t = cm.__enter__()
        ctxs.append(cm)
        return t

    def ps(name, shape):
        cm = nc.psum_tensor(name, shape, F32)
        t = cm.__enter__()
        ctxs.append(cm)
        return t

    def sem(name):
        cm = nc.semaphore(name)
        s = cm.__enter__()
        ctxs.append(cm)
        return s

    ca = sb("ca", [128, TOTA])
    cbb = sb("cbb", [128, TOTB])
    zero_sb = sb("zero_sb", [128, 2 * NH])
    warm_sb = sb("warm_sb", [128, 1], F32)
    bh4_sb = sb("bh4_sb", [128, 1], F32)
    q_sb = sb("q_sb", [128, CHUNK])
    k_sb = sb("k_sb", [128, NH])
    kst = sb("kst", [128, NH])
    s_sb = [sb(f"s{i}", [128, BPC * 128]) for i in range(2 * NCC)]
    h_sb = [sb(f"h{i}", [128, BPC * 128]) for i in range(2 * NCC)]
    pT = sb("pT", [128, 2, W])
    pTm = sb("pTm", [128, 2, W])
    ssum = sb("ssum", [128, 2], F32)
    rcp = sb("rcp", [128, 2], F32)
    dj = [[sb(f"dj{it}_{w}", [128, 128]) for w in range(2)] for it in range(2)]
    o_sb = [sb(f"o{it}", [128, D], F32) for it in range(2)]

    # separate PSUM tensors padded to a full bank (512 f32 cols) so PE writes
    # and ACT/DVE reads of different logical tensors never share a bank
    q_ps = ps("q_ps", [128, 512])
    k_ps = ps("k_ps", [128, 512])
    et0_ps = ps("et0_ps", [128, 512])
    et1_ps = ps("et1_ps", [128, 512])
    v0_ps = ps("v0_ps", [128, 512])
    v1_ps = ps("v1_ps", [128, 512])
    et_ps = [et0_ps, et1_ps]
    v_ps = [v0_ps, v1_ps]

    S_ba = sem("S_ba")      # blob A loaded
    S_bb = sem("S_bb")      # blob B loaded
    S_zero = sem("S_zero")  # dense zeroed
    S_qk = sem("S_qk")      # q (1), k (2) projections done
    S_bh = sem("S_bh")      # bh staged on DVE
    S_s = sem("S_s")        # S-chunks (8 total)
    S_h = sem("S_h")        # tanh chunks (8 total)
    S_et = sem("S_et")      # per-chunk Wa-matmul groups (8 total)
    S_praw = sem("S_praw")  # exp halves (2)
    S_pt = sem("S_pt")      # masked P halves (2)
    S_diag = sem("S_diag")  # diag scatter halves (16, 32)
    S_dj = sem("S_dj")      # transposes (16 each, 64 total)
    S_v = sem("S_v")        # value matmul halves (2)
    S_o = sem("S_o")        # scaled outputs (2)
    S_out = sem("S_out")    # stores (16, 32)

    xT_t = [ca[:, A_XT + NH * t:A_XT + NH * (t + 1)] for t in range(2)]
    wt_t = [ca[:, A_WT + 128 * t:A_WT + 128 * (t + 1)] for t in range(2)]
    wx_t = [ca[:, A_WX + 128 * t:A_WX + 128 * (t + 1)] for t in range(2)]
    xh_t = [cbb[:, B_XH + D * t:B_XH + D * (t + 1)] for t in range(3)]
    wa4 = cbb[:, B_WA:B_WA + G]
    mask_t = [cbb[:, B_MASK + W * t:B_MASK + W * (t + 1)] for t in range(2)]
    bh4_raw = cbb[:, B_BH:B_BH + 2].bitcast(F32)

    with nc.Block() as block:

        @block.sync
        def _(sync):
            sync.dma_start(out=ca[:], in_=blobA[:]).then_inc(S_ba, 16)
            for it in range(2):
                sync.wait_ge(S_diag, 16 * (it + 1))
                for w in range(2):
                    sync.dma_start_transpose(
                        out=dj[it][w][:],
                        in_=dense[128 * it:128 * (it + 1),
                                  128 * (it + w):128 * (it + w + 1)],
                    ).then_inc(S_dj, 16)

        @block.scalar
        def _(act):
            # issue blob B on the ACT HWDGE ring (parallel with blob A)
            act.dma_start(out=cbb[:], in_=blobB[:]).then_inc(S_bb, 16)
            # preload tanh/exp spline tables while DMAs fly (inputs are
            # whatever SBUF holds; outputs discarded)
            act.activation(out=warm_sb[:], in_=zero_sb[:, 0:1],
                           func=mybir.ActivationFunctionType.Tanh)
            act.activation(out=warm_sb[:], in_=zero_sb[:, 0:1],
                           func=mybir.ActivationFunctionType.Exp)
            act.wait_ge(S_bh, 1)
            for it in range(2):
                for cc in range(NCC):
                    i = NCC * it + cc
                    act.wait_ge(S_s, i + 1)
                    act.activation(out=h_sb[i][:], in_=s_sb[i][:],
                                   func=mybir.ActivationFunctionType.Tanh,
                                   bias=bh4_sb[:]).then_inc(S_h, 1)
                act.wait_ge(S_et, NCC * (it + 1))
                act.activation(out=pT[:, it, :], in_=et_ps[it][:, 0:W],
                               func=mybir.ActivationFunctionType.Exp
                               ).then_inc(S_praw, 1)

        @block.gpsimd
        def _(gpsimd):
            gpsimd.memset(zero_sb[:], 0.0)
            dense_zap = bass.AP(tensor=dense, offset=0,
                                ap=[[2 * NH, 128], [1, 2 * NH]])
            gpsimd.dma_start(out=dense_zap, in_=zero_sb[:]).then_inc(S_zero, 16)
            gpsimd.wait_ge(S_zero, 16)
            for it in range(2):
                gpsimd.wait_ge(S_pt, it + 1)
                # scatter band: dense row (128*it + p), cols [i, i+W) <- pTm
                diag = bass.AP(tensor=dense, offset=(NH + 1) * 128 * it,
                               ap=[[NH + 1, 128], [1, W]])
                gpsimd.dma_start(out=diag,
                                 in_=pTm[:, it, :]).then_inc(S_diag, 16)
            # stores: only after ALL transposes (xbar copy/transpose hazard)
            gpsimd.wait_ge(S_dj, 64)
            for it in range(2):
                gpsimd.wait_ge(S_o, it + 1)
                gpsimd.dma_start(out=out[128 * it:128 * (it + 1), :],
                                 in_=o_sb[it][:]).then_inc(S_out, 16)
            gpsimd.wait_ge(S_out, 32)

        @block.tensor
        def _(pe):
            pe.wait_ge(S_ba, 16)
            pe.matmul(q_ps[:, 0:CHUNK], lhsT=wt_t[0], rhs=xT_t[0][:, 64:64 + CHUNK],
                      start=True, stop=False)
            pe.matmul(q_ps[:, 0:CHUNK], lhsT=wt_t[1], rhs=xT_t[1][:, 64:64 + CHUNK],
                      start=False, stop=True).then_inc(S_qk, 1)
            pe.matmul(k_ps[:, 0:NH], lhsT=wx_t[0], rhs=xT_t[0], start=True, stop=False)
            pe.matmul(k_ps[:, 0:NH], lhsT=wx_t[1], rhs=xT_t[1],
                      start=False, stop=True).then_inc(S_qk, 1)
            pe.wait_ge(S_bb, 16)  # wa4 / xh
            # ET[i, 4b+r] = sum_u h[(r,u), i] * Wa4[(r,u), r]
            for it in range(2):
                for cc in range(NCC):
                    i = NCC * it + cc
                    pe.wait_ge(S_h, i + 1)
                    for t in range(BPC):
                        b = BPC * cc + t
                        mm = pe.matmul(
                            et_ps[it][:, G * b:G * (b + 1)],
                            lhsT=h_sb[i][:, 128 * t:128 * (t + 1)],
                            rhs=wa4, start=True, stop=True)
                    mm.then_inc(S_et, 1)
            for it in range(2):
                pe.wait_ge(S_dj, 32 * (it + 1))
                for w in range(2):
                    mm = pe.matmul(v_ps[it][:, 0:D], lhsT=dj[it][w][:],
                                   rhs=xh_t[it + w], start=(w == 0), stop=(w == 1))
                mm.then_inc(S_v, 1)

        @block.vector
        def _(dve):
            dve.wait_ge(S_bb, 16)
            dve.tensor_copy(out=bh4_sb[:], in_=bh4_raw).then_inc(S_bh, 1)
            dve.wait_ge(S_qk, 2)
            dve.tensor_copy(out=q_sb[:], in_=q_ps[:, 0:CHUNK])
            dve.tensor_copy(out=k_sb[:], in_=k_ps[:, 0:NH])
            for r in range(G):
                dve.tensor_copy(out=kst[32 * r:32 * (r + 1), 0:381],
                                in_=k_sb[32 * r:32 * (r + 1), r:r + 381])
            q_ap = q_sb[:]
            kst_ap = kst[:]
            for it in range(2):
                for cc in range(NCC):
                    i = NCC * it + cc
                    in0 = bass.AP(tensor=kst_ap.tensor,
                                  offset=kst_ap.offset + 128 * it + G * BPC * cc,
                                  ap=[kst_ap.ap[0], [G, BPC], [1, 128]])
                    in1 = bass.AP(tensor=q_ap.tensor, offset=q_ap.offset + 128 * it,
                                  ap=[q_ap.ap[0], [0, BPC], [1, 128]])
                    s3 = bass.AP(tensor=s_sb[i], offset=0,
                                 ap=[[BPC * 128, 128], [128, BPC], [1, 128]])
                    dve.tensor_add(out=s3, in0=in0, in1=in1).then_inc(S_s, 1)
            for it in range(2):
                dve.wait_ge(S_praw, it + 1)
                dve.tensor_mul(out=pTm[:, it, :], in0=pT[:, it, :],
                               in1=mask_t[it]).then_inc(S_pt, 1)
                dve.reduce_sum(out=ssum[:, it:it + 1], in_=pTm[:, it, :],
                               axis=mybir.AxisListType.X)
                dve.tensor_scalar_add(out=ssum[:, it:it + 1], in0=ssum[:, it:it + 1],
                                      scalar1=float(EPS))
                dve.reciprocal(out=rcp[:, it:it + 1], in_=ssum[:, it:it + 1])
            for it in range(2):
                dve.wait_ge(S_v, it + 1)
                dve.tensor_scalar_mul(out=o_sb[it][:], in0=v_ps[it][:, 0:D],
                                      scalar1=rcp[:, it:it + 1]).then_inc(S_o, 1)

    for cm in reversed(ctxs):
        cm.__exit__(None, None, None)
    return nc


def make_in_maps(x, Wt, Wx, bh, Wa, ba):
    x = np.asarray(x, np.float32)
    Wt = np.asarray(Wt, np.float32)
    Wx = np.asarray(Wx, np.float32)
    bh = np.asarray(bh, np.float32).reshape(U)
    Wa = np.asarray(Wa, np.float32).reshape(U)
    ba = np.asarray(ba, np.float32).reshape(1)

    wt4 = np.tile(Wt, (1, G)).astype(NPBF)          # [D, 128]
    wx4 = np.tile(Wx, (1, G)).astype(NPBF)
    wa4 = np.zeros((128, G), np.float32)
    for r in range(G):
        wa4[32 * r:32 * (r + 1), r] = Wa
    wa4 = wa4.astype(NPBF)
    bh4 = np.ascontiguousarray(np.tile(bh, G).reshape(128, 1), np.float32)
    bh4_bits = bh4.view(np.uint16).view(NPBF)       # [128, 2] raw f32 bytes

    dd = np.arange(W)[None, :]
    ii = np.arange(CHUNK)[:, None]

    in_maps = []
    for core in range(8):
        b, ch = divmod(core, 4)
        lo = ch * CHUNK - 64
        xpad = np.zeros((NH, D), np.float32)
        s0, s1 = max(0, lo), min(L, lo + NH)
        xpad[s0 - lo:s1 - lo] = x[b, s0:s1]
        j = lo + ii + dd
        m = (((j >= 0) & (j < L)).astype(np.float32) * np.exp(ba[0])).astype(NPBF)
        xT = np.ascontiguousarray(xpad.T).astype(NPBF)   # [D, NH]
        xh = xpad.astype(NPBF)                           # [NH, D]

        ba_arr = np.zeros((128, TOTA), NPBF)
        for t in range(2):
            ba_arr[:, A_XT + NH * t:A_XT + NH * (t + 1)] = xT[128 * t:128 * (t + 1)]
            ba_arr[:, A_WT + 128 * t:A_WT + 128 * (t + 1)] = wt4[128 * t:128 * (t + 1)]
            ba_arr[:, A_WX + 128 * t:A_WX + 128 * (t + 1)] = wx4[128 * t:128 * (t + 1)]
        bb_arr = np.zeros((128, TOTB), NPBF)
        for t in range(3):
            bb_arr[:, B_XH + D * t:B_XH + D * (t + 1)] = xh[128 * t:128 * (t + 1)]
        bb_arr[:, B_WA:B_WA + G] = wa4
        for t in range(2):
            bb_arr[:, B_MASK + W * t:B_MASK + W * (t + 1)] = m[128 * t:128 * (t + 1)]
        bb_arr[:, B_BH:B_BH + 2] = bh4_bits
        in_maps.append({"blobA": ba_arr, "blobB": bb_arr})
    return in_maps


def assemble(results):
    out = np.zeros((B, L, D), np.float32)
    for core in range(8):
        b, ch = divmod(core, 4)
        out[b, ch * CHUNK:(ch + 1) * CHUNK, :] = results[core]["out"]
    return out


def kernel(x, Wt, Wx, bh, Wa, ba):
    nc = build_bass()
    in_maps = make_in_maps(x, Wt, Wx, bh, Wa, ba)
    res = run_bass_kernel_spmd(nc, in_maps, core_ids=list(range(8)))
    return assemble(res.results)


if __name__ == "__main__":
    rng = np.random.default_rng(0)
    glorot = lambda shape: rng.standard_normal(shape, np.float32) * np.sqrt(2.0 / (shape[0] + shape[-1]))
    inputs = {
        "x": rng.standard_normal((B, L, D), np.float32),
        "Wt": glorot((D, U)), "Wx": glorot((D, U)),
        "bh": np.zeros(U, np.float32), "Wa": glorot((U, 1)),
        "ba": np.zeros(1, np.float32),
    }
    out = kernel(**inputs)
    print("kernel ran, out shape", out.shape, "finite:", np.isfinite(out).all())
